# revision 22
# baseline (speedup 1.0000x reference)
"""DeepONet (branch_v / branch_mu / trunk + Jacobians) Trainium2 kernel.

Data-parallel over batch b: 8 samples -> 8 NeuronCores, one sample per core.
MLP weights replicated; per-core inputs are the sample's slices.

Math per sample (J=128 latent, m=100 samples, 512 trunk points):
  branch_v : 200->128x4 tanh MLP; forward as PE mat-vecs (v and mu fused
             into one [128,2] lane pair per layer, biases folded into the
             matmul via a bias-row x 1.0 accumulation); input Jacobian via
             G1 = d1 (.) W1^T ; G_{k+1} = d_{k+1} (.) (W_{k+1}^T @ G_k).
  branch_mu: 100->128x4, same scheme.
  trunk    : 2->128x4 over 512 points, activations transposed
             [feature, point]; d/dx,d/dy via forward JVP with the layer-1
             tangent folded into host-prescaled W2 copies (w2tx/w2ty).
  outputs  : lin_w contractions collapse to per-partition scale vectors
             qv/qmu/c + a [128,301] rhs (A_vx|A_vy|A_mu|c) hit by 4 chunk
             matmuls (column 300 = basis), plus 8 tiny matmuls for
             db_x/db_y columns.

Toolchain constraints shaping the structure:
  - fp32r matmuls (1 cycle/row at N>=256, vs 4 for fp32) need operands
    produced as dt.float32r (dram params + tiles carry the dtype).
  - Every matmul / DMA instruction here tolerates only ONE sync-wait, so:
    inputs arrive as 3 packed single-DMA tensors, outputs leave as one
    packed DMA whose SBUF source is written by a single engine (DVE),
    tiny absorber matmuls pre-observe DMA semaphores on PE, and engine
    assignment keeps each matmul's remaining deps on one semaphore.
  - The kernel-tail drain waits one slot per distinct semaphore: 4 DMA
    lanes + PE + ACT + DVE = 7 total.
"""

import numpy as np
from contextlib import ExitStack

import concourse.bass as bass
import concourse.mybir as mybir
import concourse.tile as tile
from concourse.bass_utils import run_bass_kernel_spmd

F32 = mybir.dt.float32
F32R = mybir.dt.float32r
MULT = mybir.AluOpType.mult
ADD = mybir.AluOpType.add
TANH = mybir.ActivationFunctionType.Tanh
SQUARE = mybir.ActivationFunctionType.Square
COPY = mybir.ActivationFunctionType.Copy

B, M, E, I, J = 8, 100, 128, 4, 128
NPT = E * I          # 512 trunk points per sample
DINV = 2 * M         # 200

TRACE = False
LAST_RESULT = None
_NC = None

# ---- pkB (f32r, [2, 1153]): trunk input/L1 weight + branch bias rows ----
B_XT, B_W1T, B_BROWV, B_BROWM, B_ONE = 0, 512, 640, 1152, 1664
CB = 1665
# ---- pkE (f32, [128, 657]): branch early weights/inputs + biases ----
E_XVA, E_XVB, E_XMU = 0, 1, 2
E_W1VA, E_W1VB, E_W1MU = 3, 131, 259
E_W2V, E_W2MU = 387, 515
E_BT, E_LINW, E_ZERO = 643, 647, 648
CE = 649
# ---- pkT (f32r, [128, 640]): trunk weights ----
L_W2T, L_W3T, L_W4T, L_W2TX, L_W2TY = 0, 128, 256, 384, 512
CT = 640
# ---- pkV (f32, [128, 812]): late branch weights (exact fp32) ----
V_W3V, V_W4V, V_W3MU, V_W4MU = 0, 128, 256, 384
V_W1VT, V_W1MUT = 512, 712
CV = 812
# ---- o_all (f32, [128, 1216]): 4 x [vx|vy|mu|basis] + x cols + y cols ----
O_CH = 302            # vx(100) | vy(100) | mu(100) | basis | pad (N even for fp32r)
O_X, O_Y = 1208, 1212
CO = 1216


def _emit(nc, tc, D, O):
    with ExitStack() as ctx:
        wp = ctx.enter_context(tc.tile_pool(name="w", bufs=1))
        sp = ctx.enter_context(tc.tile_pool(name="s", bufs=1))
        tp = ctx.enter_context(tc.tile_pool(name="t", bufs=2))
        pph = ctx.enter_context(tc.tile_pool(name="pph", bufs=1, space="PSUM"))
        ppj = ctx.enter_context(tc.tile_pool(name="ppj", bufs=2, space="PSUM"))
        ppg = ctx.enter_context(tc.tile_pool(name="ppg", bufs=2, space="PSUM"))
        ppo = ctx.enter_context(tc.tile_pool(name="ppo", bufs=2, space="PSUM"))
        ppc = ctx.enter_context(tc.tile_pool(name="ppc", bufs=1, space="PSUM"))

        pkB = wp.tile([2, CB], F32R, tag="pkB")
        nc.sync.dma_start(pkB[:], D["pkB"][:])
        pkE = wp.tile([128, CE], F32, tag="pkE")
        nc.sync.dma_start(pkE[:], D["pkE"][:])
        pkT = wp.tile([128, CT], F32R, tag="pkT")
        nc.sync.dma_start(pkT[:], D["pkT"][:])
        pkV = wp.tile([128, CV], F32, tag="pkV")
        nc.sync.dma_start(pkV[:], D["pkV"][:])

        OSB = sp.tile([128, CO], F32, tag="osb")

        mm = nc.tensor.matmul

        # ACT/DVE observe the pkE DMA semaphore before any real consumer,
        # so later ops keep a single unobserved dependency each.
        scr_a = sp.tile([1, 1], F32, tag="scr_a")
        nc.scalar.activation(scr_a[:], pkE[0:1, 0:1], COPY, bias=0.0)
        scr_d = sp.tile([1, 1], F32, tag="scr_d")
        nc.vector.tensor_copy(scr_d[:], pkE[0:1, 0:1])

        def eco(c0, n=1, p=128, p0=0):
            return pkE[p0:p0 + p, c0:c0 + n]

        def vco(c0, n=128):
            return pkV[:, c0:c0 + n]

        zero_b = eco(E_ZERO)

        # branch layer step. The v-column group must fully close before the
        # mu-column group opens: an interleaved start=True clears the whole
        # psum bank, not just its own elements.
        def bstep(pm, k, mu_ops):
            mm(pm[:, 0:1],
               pkB[0:2, B_BROWV + 128 * (k - 1):B_BROWV + 128 * k].bitcast(F32),
               pkB[0:2, B_ONE:B_ONE + 1].bitcast(F32), start=False, stop=True)
            mm(pm[:, 1:2], mu_ops[0], mu_ops[1], start=True, stop=False)
            mm(pm[:, 1:2],
               pkB[0:2, B_BROWM + 128 * (k - 1):B_BROWM + 128 * k].bitcast(F32),
               pkB[0:2, B_ONE:B_ONE + 1].bitcast(F32), start=False, stop=True)
            h = sp.tile([128, 2], F32, tag=f"hb{k}")
            nc.scalar.activation(h[:], pm[:, 0:2], TANH, bias=zero_b)
            sq = tp.tile([128, 2], F32, tag="sqb")
            nc.scalar.activation(sq[:], h[:], SQUARE, bias=zero_b)
            d = sp.tile([128, 2], F32, tag=f"db{k}")
            nc.vector.tensor_scalar(d[:], sq[:], -1.0, 1.0, MULT, ADD)
            return h, d

        hb, db = {}, {}
        # one psum bank holds: branch matvec cols 0:8, absorber cols 8:10,
        # db_x/db_y cols 10:18 -- all disjoint, never re-slotted
        pbr = ppc.tile([128, 18], F32, tag="pc")

        # ---- PE ladder start: branch L1 (pkE) + trunk L1 (pkB) ----
        pm = pbr[:, 0:2]
        mm(pm[:, 0:1], eco(E_W1VA, 128), eco(E_XVA), start=True, stop=False)
        mm(pm[:, 0:1], eco(E_W1VB, 128, p=72), eco(E_XVB, p=72),
           start=False, stop=False)

        ph = pph.tile([128, NPT], F32, tag="ph")
        mm(ph[:], pkB[0:2, B_W1T:B_W1T + 128], pkB[0:2, B_XT:B_XT + NPT],
           start=True, stop=True)

        hb[1], db[1] = bstep(pm, 1,
                             (eco(E_W1MU, 128, p=100), eco(E_XMU, p=100)))

        h1t = sp.tile([128, NPT], F32R, tag="h1t")
        nc.scalar.activation(h1t[:], ph[:], TANH, bias=eco(E_BT + 0))
        sq = tp.tile([128, NPT], F32, tag="sqt")
        nc.scalar.activation(sq[:], h1t[:], SQUARE, bias=zero_b)
        d1t = sp.tile([128, NPT], F32R, tag="d1t")
        nc.vector.tensor_scalar(d1t[:], sq[:], -1.0, 1.0, MULT, ADD)

        # ---- branch L2 (weights in pkE) ----
        pm = pbr[:, 2:4]
        mm(pm[:, 0:1], eco(E_W2V, 128), hb[1][:, 0:1], start=True, stop=False)
        hb[2], db[2] = bstep(pm, 2, (eco(E_W2MU, 128), hb[1][:, 1:2]))

        # ---- absorber for pkL (ldweights: no psum, 1 wait), then trunk L2 ----
        nc.tensor.ldweights(pkT[:, 0:1].bitcast(mybir.dt.bfloat16))

        ph = pph.tile([128, NPT], F32, tag="ph")
        mm(ph[:], pkT[:, L_W2T:L_W2T + 128], h1t[:], start=True, stop=True)
        h2t = sp.tile([128, NPT], F32R, tag="h2t")
        nc.scalar.activation(h2t[:], ph[:], TANH, bias=eco(E_BT + 1))
        sq = tp.tile([128, NPT], F32, tag="sqt")
        nc.scalar.activation(sq[:], h2t[:], SQUARE, bias=zero_b)
        d2t = sp.tile([128, NPT], F32, tag="d2t")
        nc.vector.tensor_scalar(d2t[:], sq[:], -1.0, 1.0, MULT, ADD)

        prx = ppj.tile([128, NPT], F32, tag="pj")
        mm(prx[:], pkT[:, L_W2TX:L_W2TX + 128], d1t[:], start=True, stop=True)
        u2x = sp.tile([128, NPT], F32R, tag="u2x")
        nc.vector.tensor_tensor(u2x[:], prx[:], d2t[:], MULT)
        pry = ppj.tile([128, NPT], F32, tag="pj")
        mm(pry[:], pkT[:, L_W2TY:L_W2TY + 128], d1t[:], start=True, stop=True)
        u2y = sp.tile([128, NPT], F32R, tag="u2y")
        nc.vector.tensor_tensor(u2y[:], pry[:], d2t[:], MULT)

        # ---- branch L3/L4 (weights in pkL) + trunk L3/L4 interleaved ----
        tr_w = {3: L_W3T, 4: L_W4T}
        br_w = {3: (V_W3V, V_W3MU), 4: (V_W4V, V_W4MU)}
        hprev, ux, uy = h2t, u2x, u2y
        h4t = u4x = u4y = None
        for k in (3, 4):
            wv_c, wmu_c = br_w[k]
            pm = pbr[:, 2 * k - 2:2 * k]
            mm(pm[:, 0:1], vco(wv_c), hb[k - 1][:, 0:1], start=True, stop=False)
            hb[k], db[k] = bstep(pm, k, (vco(wmu_c), hb[k - 1][:, 1:2]))

            wc = tr_w[k]
            ph = pph.tile([128, NPT], F32, tag="ph")
            mm(ph[:], pkT[:, wc:wc + 128], hprev[:], start=True, stop=True)
            h = sp.tile([128, NPT], F32R, tag=f"h{k}t")
            nc.scalar.activation(h[:], ph[:], TANH, bias=eco(E_BT + k - 1))
            sq = tp.tile([128, NPT], F32, tag="sqt")
            nc.scalar.activation(sq[:], h[:], SQUARE, bias=zero_b)
            d = sp.tile([128, NPT], F32, tag=f"d{k}t")
            nc.vector.tensor_scalar(d[:], sq[:], -1.0, 1.0, MULT, ADD)

            prx = ppj.tile([128, NPT], F32, tag="pj")
            mm(prx[:], pkT[:, wc:wc + 128], ux[:], start=True, stop=True)
            nux = sp.tile([128, NPT], F32R, tag=f"u{k}x")
            nc.vector.tensor_tensor(nux[:], prx[:], d[:], MULT)
            pry = ppj.tile([128, NPT], F32, tag="pj")
            mm(pry[:], pkT[:, wc:wc + 128], uy[:], start=True, stop=True)
            nuy = sp.tile([128, NPT], F32R, tag=f"u{k}y")
            nc.vector.tensor_tensor(nuy[:], pry[:], d[:], MULT)
            hprev, ux, uy = h, nux, nuy
            if k == 4:
                h4t, u4x, u4y = h, nux, nuy

        # ---- scale vectors on DVE (emitted before ABC writes) ----
        linw = eco(E_LINW)
        qv = sp.tile([128, 1], F32, tag="qv")
        nc.vector.scalar_tensor_tensor(qv[:], hb[4][:, 1:2], linw, db[4][:, 0:1],
                                       MULT, MULT)
        qmu = sp.tile([128, 1], F32, tag="qmu")
        nc.vector.scalar_tensor_tensor(qmu[:], hb[4][:, 0:1], linw, db[4][:, 1:2],
                                       MULT, MULT)
        c = sp.tile([128, 1], F32R, tag="c")
        nc.vector.scalar_tensor_tensor(c[:], hb[4][:, 0:1], linw, hb[4][:, 1:2],
                                       MULT, MULT)

        # ---- branch Jacobian chains -> ABC [128, 301] ----
        ABC = sp.tile([128, 3 * M + 2], F32R, tag="abc")
        nc.vector.tensor_copy(ABC[:, 3 * M:3 * M + 1], c[:])
        nc.vector.tensor_copy(ABC[:, 3 * M + 1:3 * M + 2], c[:])

        def gchain(w1t_c, n, dcol, w2_ap, w3_c, w4_c, q, c0):
            g = sp.tile([128, n], F32, tag=f"g{c0}")
            nc.vector.tensor_scalar_mul(g[:], vco(w1t_c, n), db[1][:, dcol:dcol + 1])
            pg = ppg.tile([128, DINV], F32, tag="pg")
            mm(pg[:, 0:n], w2_ap, g[:], start=True, stop=True)
            g2 = sp.tile([128, n], F32, tag=f"g2{c0}")
            nc.vector.tensor_scalar_mul(g2[:], pg[:, 0:n], db[2][:, dcol:dcol + 1])
            pg = ppg.tile([128, DINV], F32, tag="pg")
            mm(pg[:, 0:n], vco(w3_c), g2[:], start=True, stop=True)
            g3 = sp.tile([128, n], F32, tag=f"g3{c0}")
            nc.vector.tensor_scalar_mul(g3[:], pg[:, 0:n], db[3][:, dcol:dcol + 1])
            pg = ppg.tile([128, DINV], F32, tag="pg")
            mm(pg[:, 0:n], vco(w4_c), g3[:], start=True, stop=True)
            nc.vector.tensor_scalar_mul(ABC[:, c0:c0 + n], pg[:, 0:n], q[:])

        gchain(V_W1VT, DINV, 0, eco(E_W2V, 128), V_W3V, V_W4V, qv, 0)
        gchain(V_W1MUT, M, 1, eco(E_W2MU, 128), V_W3MU, V_W4MU, qmu, 2 * M)

        # ---- outputs ----
        # absorber: PE observes ACT tick of h4t
        nc.tensor.ldweights(h4t[:, 0:1].bitcast(mybir.dt.bfloat16))
        for ci in range(4):
            po = ppo.tile([128, NPT], F32, tag="po")
            mm(po[:, 0:3 * M + 2], h4t[:, ci * 128:(ci + 1) * 128], ABC[:],
               start=True, stop=True)
            nc.vector.tensor_copy(OSB[:, O_CH * ci:O_CH * (ci + 1)],
                                  po[:, 0:3 * M + 2])
            nc.sync.dma_start(O["o_all"][:, O_CH * ci:O_CH * (ci + 1)],
                              OSB[:, O_CH * ci:O_CH * (ci + 1)])
        # db_x / db_y columns: 8 tiny matmuls into one psum, one DVE copy
        for ci in range(4):
            mm(pbr[:, 10 + ci:11 + ci],
               u4x[:, ci * 128:(ci + 1) * 128].bitcast(F32), c[:].bitcast(F32),
               start=True, stop=True)
            mm(pbr[:, 14 + ci:15 + ci],
               u4y[:, ci * 128:(ci + 1) * 128].bitcast(F32), c[:].bitcast(F32),
               start=True, stop=True)
        nc.vector.tensor_copy(OSB[:, O_X:O_X + 8], pbr[:, 10:18])
        nc.sync.dma_start(O["o_all"][:, O_X:O_X + 8], OSB[:, O_X:O_X + 8])


def _split_multiwaits(nc):
    """Walrus codegen accepts a single sync-wait per instruction; Tile can
    emit several. Hoist all but the last wait onto same-engine NoOps
    inserted immediately before the over-subscribed instruction."""
    k = 0
    for fn in nc.m.functions:
        for bb in fn.blocks:
            il = bb.instructions
            out = []
            changed = False
            for inst in il:
                si = inst.sync_info
                waits = list(si.on_wait) if si and si.on_wait else []
                if len(waits) > 1:
                    changed = True
                    for w in waits[:-1]:
                        k += 1
                        nop = mybir.InstNoOp(name=f"I-wsplit-{k}")
                        nop.engine = inst.engine
                        nop.sync_info = mybir.SyncInfo(on_wait=[w], on_update=[])
                        nc.register_instruction(nop)
                        out.append(nop)
                    inst.sync_info = mybir.SyncInfo(
                        on_wait=[waits[-1]], on_update=list(si.on_update or []))
                out.append(inst)
            if changed:
                il[:] = out
                assert [i.name for i in bb.instructions] == [i.name for i in out]
    return k


def _build_nc():
    nc = bass.Bass()
    dp = nc.declare_dram_parameter
    D = {
        "pkB": dp("pkB", [2, CB], F32R, isOutput=False),
        "pkE": dp("pkE", [128, CE], F32, isOutput=False),
        "pkT": dp("pkT", [128, CT], F32R, isOutput=False),
        "pkV": dp("pkV", [128, CV], F32, isOutput=False),
    }
    O = {"o_all": dp("o_all", [128, CO], F32, isOutput=True)}
    with tile.TileContext(nc) as tc:
        _emit(nc, tc, D, O)
    _split_multiwaits(nc)
    return nc


def _prep_core_inputs(v_x, v_y, DT, cx, cy, brv, brmu, tr, lin_w):
    f = np.float32
    xv = np.concatenate([v_x, v_y], axis=2).reshape(B, DINV).astype(f)
    xt_x = cx.reshape(B, -1).astype(f)
    xt_y = cy.reshape(B, -1).astype(f)

    W1v = np.asarray(brv[0][0], f)
    W1mu = np.asarray(brmu[0][0], f)
    W1t = np.asarray(tr[0][0], f)
    perm = np.concatenate([np.arange(0, DINV, 2), np.arange(1, DINV, 2)])

    pkB = np.zeros((2, CB), f)
    pkB[0:2, B_W1T:B_W1T + 128] = W1t
    for k in range(4):
        pkB[0, B_BROWV + 128 * k:B_BROWV + 128 * (k + 1)] = np.asarray(brv[k][1], f)
        pkB[0, B_BROWM + 128 * k:B_BROWM + 128 * (k + 1)] = np.asarray(brmu[k][1], f)
    pkB[0, B_ONE] = 1.0

    pkE = np.zeros((128, CE), f)
    pkE[0:128, E_W1VA:E_W1VA + 128] = W1v[0:128]
    pkE[0:72, E_W1VB:E_W1VB + 128] = W1v[128:200]
    pkE[0:100, E_W1MU:E_W1MU + 128] = W1mu
    pkE[:, E_W2V:E_W2V + 128] = brv[1][0]
    pkE[:, E_W2MU:E_W2MU + 128] = brmu[1][0]
    for k in range(4):
        pkE[:, E_BT + k] = np.asarray(tr[k][1], f)
    pkE[:, E_LINW] = np.asarray(lin_w, f).reshape(J)

    pkT = np.zeros((128, CT), f)
    pkT[:, L_W2T:L_W2T + 128] = tr[1][0]
    pkT[:, L_W3T:L_W3T + 128] = tr[2][0]
    pkT[:, L_W4T:L_W4T + 128] = tr[3][0]
    pkT[:, L_W2TX:L_W2TX + 128] = np.asarray(tr[1][0], f) * W1t[0][:, None]
    pkT[:, L_W2TY:L_W2TY + 128] = np.asarray(tr[1][0], f) * W1t[1][:, None]
    pkV = np.zeros((128, CV), f)
    pkV[:, V_W3V:V_W3V + 128] = brv[2][0]
    pkV[:, V_W4V:V_W4V + 128] = brv[3][0]
    pkV[:, V_W3MU:V_W3MU + 128] = brmu[2][0]
    pkV[:, V_W4MU:V_W4MU + 128] = brmu[3][0]
    pkV[:, V_W1VT:V_W1VT + DINV] = W1v.T[:, perm]
    pkV[:, V_W1MUT:V_W1MUT + M] = W1mu.T

    in_maps = []
    for b in range(B):
        mB = pkB.copy()
        mB[0, B_XT:B_XT + NPT] = xt_x[b]
        mB[1, B_XT:B_XT + NPT] = xt_y[b]
        mE = pkE.copy()
        mE[0:128, E_XVA] = xv[b, 0:128]
        mE[0:72, E_XVB] = xv[b, 128:200]
        mE[0:100, E_XMU] = DT[b].astype(f)
        in_maps.append({"pkB": mB, "pkE": mE, "pkT": pkT, "pkV": pkV})
    return in_maps


def kernel(v_x_sampled, v_y_sampled, DT_sampled, coord_x, coord_y,
           brv_params, brmu_params, tr_params, lin_w):
    global _NC, LAST_RESULT
    a = np.asarray
    brv = [(a(W, dtype=np.float32), a(bb, dtype=np.float32)) for W, bb in brv_params]
    brmu = [(a(W, dtype=np.float32), a(bb, dtype=np.float32)) for W, bb in brmu_params]
    tr = [(a(W, dtype=np.float32), a(bb, dtype=np.float32)) for W, bb in tr_params]

    in_maps = _prep_core_inputs(
        a(v_x_sampled, dtype=np.float32), a(v_y_sampled, dtype=np.float32),
        a(DT_sampled, dtype=np.float32),
        a(coord_x, dtype=np.float32), a(coord_y, dtype=np.float32),
        brv, brmu, tr, a(lin_w, dtype=np.float32))

    if _NC is None:
        _NC = _build_nc()

    res = run_bass_kernel_spmd(_NC, in_maps, list(range(B)), trace=TRACE)
    LAST_RESULT = res

    def unpack(r):
        o = r["o_all"]
        vx = np.concatenate([o[:, O_CH * ci:O_CH * ci + M] for ci in range(4)])
        vy = np.concatenate([o[:, O_CH * ci + M:O_CH * ci + 2 * M]
                             for ci in range(4)])
        mu = np.concatenate([o[:, O_CH * ci + 2 * M:O_CH * ci + 3 * M]
                             for ci in range(4)])
        bas = np.concatenate([o[:, O_CH * ci + 3 * M] for ci in range(4)])
        x = np.concatenate([o[:, O_X + ci] for ci in range(4)])
        y = np.concatenate([o[:, O_Y + ci] for ci in range(4)])
        return bas, vx, vy, mu, x, y

    outs = [unpack(r) for r in res.results]
    basis = np.stack([o[0].reshape(E, I, 1) for o in outs])
    db_vx = np.stack([o[1].reshape(E, I, M, 1) for o in outs])
    db_vy = np.stack([o[2].reshape(E, I, M, 1) for o in outs])
    db_mu = np.stack([o[3].reshape(E, I, M, 1) for o in outs])
    db_x = np.stack([o[4].reshape(E, I, 1) for o in outs])
    db_y = np.stack([o[5].reshape(E, I, 1) for o in outs])
    return (basis, db_vx, db_vy, db_mu, db_x, db_y)


# revision 23
# speedup vs baseline: 1.0572x; 1.0572x over previous
"""DeepONet (branch_v / branch_mu / trunk + Jacobians) Trainium2 kernel.

Data-parallel over batch b: 8 samples -> 8 NeuronCores, one sample per core.
MLP weights replicated; per-core inputs are the sample's slices.

Math per sample (J=128 latent, m=100 samples, 512 trunk points):
  branch_v : 200->128x4 tanh MLP; forward as PE mat-vecs (v and mu fused
             into one [128,2] lane pair per layer, biases folded into the
             matmul via a bias-row x 1.0 accumulation); input Jacobian via
             G1 = d1 (.) W1^T ; G_{k+1} = d_{k+1} (.) (W_{k+1}^T @ G_k).
  branch_mu: 100->128x4, same scheme.
  trunk    : 2->128x4 over 512 points, activations transposed
             [feature, point]; d/dx,d/dy via forward JVP with the layer-1
             tangent folded into host-prescaled W2 copies (w2tx/w2ty).
  outputs  : lin_w contractions collapse to per-partition scale vectors
             qv/qmu/c + a [128,301] rhs (A_vx|A_vy|A_mu|c) hit by 4 chunk
             matmuls (column 300 = basis), plus 8 tiny matmuls for
             db_x/db_y columns.

Toolchain constraints shaping the structure:
  - fp32r matmuls (1 cycle/row at N>=256, vs 4 for fp32) need operands
    produced as dt.float32r (dram params + tiles carry the dtype).
  - Every matmul / DMA instruction here tolerates only ONE sync-wait, so:
    inputs arrive as 3 packed single-DMA tensors, outputs leave as one
    packed DMA whose SBUF source is written by a single engine (DVE),
    tiny absorber matmuls pre-observe DMA semaphores on PE, and engine
    assignment keeps each matmul's remaining deps on one semaphore.
  - The kernel-tail drain waits one slot per distinct semaphore: 4 DMA
    lanes + PE + ACT + DVE = 7 total.
"""

import numpy as np
from contextlib import ExitStack

import concourse.bass as bass
import concourse.mybir as mybir
import concourse.tile as tile
from concourse.bass_utils import run_bass_kernel_spmd

F32 = mybir.dt.float32
F32R = mybir.dt.float32r
MULT = mybir.AluOpType.mult
ADD = mybir.AluOpType.add
TANH = mybir.ActivationFunctionType.Tanh
SQUARE = mybir.ActivationFunctionType.Square
COPY = mybir.ActivationFunctionType.Copy

B, M, E, I, J = 8, 100, 128, 4, 128
NPT = E * I          # 512 trunk points per sample
DINV = 2 * M         # 200

TRACE = False
LAST_RESULT = None
_NC = None

# ---- pkB (f32r, [2, 1153]): trunk input/L1 weight + branch bias rows ----
B_XT, B_W1T, B_BROWV, B_BROWM, B_ONE = 0, 512, 640, 1152, 1664
CB = 1665
# ---- pkE (f32, [128, 657]): branch early weights/inputs + biases ----
E_XVA, E_XVB, E_XMU = 0, 1, 2
E_W1VA, E_W1VB, E_W1MU = 3, 131, 259
E_W2V, E_W2MU = 387, 515
E_BT, E_LINW, E_ZERO = 643, 647, 648
E_BV, E_BMU = 649, 653
CE = 657
# ---- pkT (f32r, [128, 640]): trunk weights ----
L_W2T, L_W3T, L_W4T, L_W2TX, L_W2TY = 0, 128, 256, 384, 512
CT = 640
# ---- pkV (f32, [128, 812]): late branch weights (exact fp32) ----
V_W3V, V_W4V, V_W3MU, V_W4MU = 0, 128, 256, 384
V_W1VT, V_W1MUT = 512, 712
CV = 812
# ---- o_all (f32, [128, 1216]): 4 x [vx|vy|mu|basis] + x cols + y cols ----
O_CH = 302            # vx(100) | vy(100) | mu(100) | basis | pad (N even for fp32r)
O_X, O_Y = 1208, 1212
CO = 1216


def _emit(nc, tc, D, O):
    with ExitStack() as ctx:
        wp = ctx.enter_context(tc.tile_pool(name="w", bufs=1))
        sp = ctx.enter_context(tc.tile_pool(name="s", bufs=1))
        tp = ctx.enter_context(tc.tile_pool(name="t", bufs=2))
        pph = ctx.enter_context(tc.tile_pool(name="pph", bufs=1, space="PSUM"))
        ppj = ctx.enter_context(tc.tile_pool(name="ppj", bufs=2, space="PSUM"))
        ppg = ctx.enter_context(tc.tile_pool(name="ppg", bufs=2, space="PSUM"))
        ppo = ctx.enter_context(tc.tile_pool(name="ppo", bufs=2, space="PSUM"))
        ppc = ctx.enter_context(tc.tile_pool(name="ppc", bufs=1, space="PSUM"))

        pkB = wp.tile([2, CB], F32R, tag="pkB")
        nc.sync.dma_start(pkB[:], D["pkB"][:])
        pkE = wp.tile([128, CE], F32, tag="pkE")
        nc.sync.dma_start(pkE[:], D["pkE"][:])
        pkT = wp.tile([128, CT], F32R, tag="pkT")
        nc.sync.dma_start(pkT[:], D["pkT"][:])
        pkV = wp.tile([128, CV], F32, tag="pkV")
        nc.sync.dma_start(pkV[:], D["pkV"][:])

        OSB = sp.tile([128, CO], F32, tag="osb")

        mm = nc.tensor.matmul

        # ACT/DVE observe the pkE DMA semaphore before any real consumer,
        # so later ops keep a single unobserved dependency each.
        scr_a = sp.tile([1, 1], F32, tag="scr_a")
        nc.scalar.activation(scr_a[:], pkE[0:1, 0:1], COPY, bias=0.0)
        scr_d = sp.tile([1, 1], F32, tag="scr_d")
        nc.vector.tensor_copy(scr_d[:], pkE[0:1, 0:1])

        def eco(c0, n=1, p=128, p0=0):
            return pkE[p0:p0 + p, c0:c0 + n]

        def vco(c0, n=128):
            return pkV[:, c0:c0 + n]

        zero_b = eco(E_ZERO)

        # branch layer step. The v-column group must fully close before the
        # mu-column group opens: an interleaved start=True clears the whole
        # psum bank, not just its own elements.
        def bstep(pm, k, mu_ops):
            mm(pm[:, 1:2], mu_ops[0], mu_ops[1], start=True, stop=True)
            h = sp.tile([128, 2], F32, tag=f"hb{k}")
            nc.scalar.activation(h[:, 0:1], pm[:, 0:1], TANH,
                                 bias=eco(E_BV + k - 1))
            nc.scalar.activation(h[:, 1:2], pm[:, 1:2], TANH,
                                 bias=eco(E_BMU + k - 1))
            sq = tp.tile([128, 2], F32, tag="sqb")
            nc.scalar.activation(sq[:], h[:], SQUARE, bias=zero_b)
            d = sp.tile([128, 2], F32, tag=f"db{k}")
            nc.vector.tensor_scalar(d[:], sq[:], -1.0, 1.0, MULT, ADD)
            return h, d

        hb, db = {}, {}
        # one psum bank holds: branch matvec cols 0:8, absorber cols 8:10,
        # db_x/db_y cols 10:18 -- all disjoint, never re-slotted
        pbr = ppc.tile([128, 18], F32, tag="pc")

        # ---- PE ladder start: branch L1 (pkE) + trunk L1 (pkB) ----
        pm = pbr[:, 0:2]
        mm(pm[:, 0:1], eco(E_W1VA, 128), eco(E_XVA), start=True, stop=False)
        mm(pm[:, 0:1], eco(E_W1VB, 128, p=72), eco(E_XVB, p=72),
           start=False, stop=True)

        ph = pph.tile([128, NPT], F32, tag="ph")
        mm(ph[:], pkB[0:2, B_W1T:B_W1T + 128], pkB[0:2, B_XT:B_XT + NPT],
           start=True, stop=True)

        hb[1], db[1] = bstep(pm, 1,
                             (eco(E_W1MU, 128, p=100), eco(E_XMU, p=100)))

        h1t = sp.tile([128, NPT], F32R, tag="h1t")
        nc.scalar.activation(h1t[:], ph[:], TANH, bias=eco(E_BT + 0))
        sq = tp.tile([128, NPT], F32, tag="sqt")
        nc.scalar.activation(sq[:], h1t[:], SQUARE, bias=zero_b)
        d1t = sp.tile([128, NPT], F32R, tag="d1t")
        nc.vector.tensor_scalar(d1t[:], sq[:], -1.0, 1.0, MULT, ADD)

        # ---- branch L2 (weights in pkE) ----
        pm = pbr[:, 2:4]
        mm(pm[:, 0:1], eco(E_W2V, 128), hb[1][:, 0:1], start=True, stop=True)
        hb[2], db[2] = bstep(pm, 2, (eco(E_W2MU, 128), hb[1][:, 1:2]))

        # ---- absorber for pkL (ldweights: no psum, 1 wait), then trunk L2 ----
        nc.tensor.ldweights(pkT[:, 0:1].bitcast(mybir.dt.bfloat16))

        ph = pph.tile([128, NPT], F32, tag="ph")
        mm(ph[:], pkT[:, L_W2T:L_W2T + 128], h1t[:], start=True, stop=True)
        h2t = sp.tile([128, NPT], F32R, tag="h2t")
        nc.scalar.activation(h2t[:], ph[:], TANH, bias=eco(E_BT + 1))
        sq = tp.tile([128, NPT], F32, tag="sqt")
        nc.scalar.activation(sq[:], h2t[:], SQUARE, bias=zero_b)
        d2t = sp.tile([128, NPT], F32, tag="d2t")
        nc.vector.tensor_scalar(d2t[:], sq[:], -1.0, 1.0, MULT, ADD)

        prx = ppj.tile([128, NPT], F32, tag="pj")
        mm(prx[:], pkT[:, L_W2TX:L_W2TX + 128], d1t[:], start=True, stop=True)
        u2x = sp.tile([128, NPT], F32R, tag="u2x")
        nc.vector.tensor_tensor(u2x[:], prx[:], d2t[:], MULT)
        pry = ppj.tile([128, NPT], F32, tag="pj")
        mm(pry[:], pkT[:, L_W2TY:L_W2TY + 128], d1t[:], start=True, stop=True)
        u2y = sp.tile([128, NPT], F32R, tag="u2y")
        nc.vector.tensor_tensor(u2y[:], pry[:], d2t[:], MULT)

        # ---- branch L3/L4 (weights in pkL) + trunk L3/L4 interleaved ----
        tr_w = {3: L_W3T, 4: L_W4T}
        br_w = {3: (V_W3V, V_W3MU), 4: (V_W4V, V_W4MU)}
        hprev, ux, uy = h2t, u2x, u2y
        h4t = u4x = u4y = None
        for k in (3, 4):
            wv_c, wmu_c = br_w[k]
            pm = pbr[:, 2 * k - 2:2 * k]
            mm(pm[:, 0:1], vco(wv_c), hb[k - 1][:, 0:1], start=True, stop=True)
            hb[k], db[k] = bstep(pm, k, (vco(wmu_c), hb[k - 1][:, 1:2]))

            wc = tr_w[k]
            ph = pph.tile([128, NPT], F32, tag="ph")
            mm(ph[:], pkT[:, wc:wc + 128], hprev[:], start=True, stop=True)
            h = sp.tile([128, NPT], F32R, tag=f"h{k}t")
            nc.scalar.activation(h[:], ph[:], TANH, bias=eco(E_BT + k - 1))
            sq = tp.tile([128, NPT], F32, tag="sqt")
            nc.scalar.activation(sq[:], h[:], SQUARE, bias=zero_b)
            d = sp.tile([128, NPT], F32, tag=f"d{k}t")
            nc.vector.tensor_scalar(d[:], sq[:], -1.0, 1.0, MULT, ADD)

            prx = ppj.tile([128, NPT], F32, tag="pj")
            mm(prx[:], pkT[:, wc:wc + 128], ux[:], start=True, stop=True)
            nux = sp.tile([128, NPT], F32R, tag=f"u{k}x")
            nc.vector.tensor_tensor(nux[:], prx[:], d[:], MULT)
            pry = ppj.tile([128, NPT], F32, tag="pj")
            mm(pry[:], pkT[:, wc:wc + 128], uy[:], start=True, stop=True)
            nuy = sp.tile([128, NPT], F32R, tag=f"u{k}y")
            nc.vector.tensor_tensor(nuy[:], pry[:], d[:], MULT)
            hprev, ux, uy = h, nux, nuy
            if k == 4:
                h4t, u4x, u4y = h, nux, nuy

        # ---- scale vectors on DVE (emitted before ABC writes) ----
        linw = eco(E_LINW)
        qv = sp.tile([128, 1], F32, tag="qv")
        nc.vector.scalar_tensor_tensor(qv[:], hb[4][:, 1:2], linw, db[4][:, 0:1],
                                       MULT, MULT)
        qmu = sp.tile([128, 1], F32, tag="qmu")
        nc.vector.scalar_tensor_tensor(qmu[:], hb[4][:, 0:1], linw, db[4][:, 1:2],
                                       MULT, MULT)
        c = sp.tile([128, 1], F32R, tag="c")
        nc.vector.scalar_tensor_tensor(c[:], hb[4][:, 0:1], linw, hb[4][:, 1:2],
                                       MULT, MULT)

        # ---- branch Jacobian chains -> ABC [128, 301] ----
        ABC = sp.tile([128, 3 * M + 2], F32R, tag="abc")
        nc.vector.tensor_copy(ABC[:, 3 * M:3 * M + 1], c[:])
        nc.vector.tensor_copy(ABC[:, 3 * M + 1:3 * M + 2], c[:])

        def gchain(w1t_c, n, dcol, w2_ap, w3_c, w4_c, q, c0):
            g = sp.tile([128, n], F32, tag=f"g{c0}")
            nc.vector.tensor_scalar_mul(g[:], vco(w1t_c, n), db[1][:, dcol:dcol + 1])
            pg = ppg.tile([128, DINV], F32, tag="pg")
            mm(pg[:, 0:n], w2_ap, g[:], start=True, stop=True)
            g2 = sp.tile([128, n], F32, tag=f"g2{c0}")
            nc.vector.tensor_scalar_mul(g2[:], pg[:, 0:n], db[2][:, dcol:dcol + 1])
            pg = ppg.tile([128, DINV], F32, tag="pg")
            mm(pg[:, 0:n], vco(w3_c), g2[:], start=True, stop=True)
            g3 = sp.tile([128, n], F32, tag=f"g3{c0}")
            nc.vector.tensor_scalar_mul(g3[:], pg[:, 0:n], db[3][:, dcol:dcol + 1])
            pg = ppg.tile([128, DINV], F32, tag="pg")
            mm(pg[:, 0:n], vco(w4_c), g3[:], start=True, stop=True)
            nc.vector.tensor_scalar_mul(ABC[:, c0:c0 + n], pg[:, 0:n], q[:])

        gchain(V_W1VT, DINV, 0, eco(E_W2V, 128), V_W3V, V_W4V, qv, 0)
        gchain(V_W1MUT, M, 1, eco(E_W2MU, 128), V_W3MU, V_W4MU, qmu, 2 * M)

        # ---- outputs ----
        # absorber: PE observes ACT tick of h4t
        nc.tensor.ldweights(h4t[:, 0:1].bitcast(mybir.dt.bfloat16))
        for ci in range(4):
            po = ppo.tile([128, NPT], F32, tag="po")
            mm(po[:, 0:3 * M + 2], h4t[:, ci * 128:(ci + 1) * 128], ABC[:],
               start=True, stop=True)
            nc.vector.tensor_copy(OSB[:, O_CH * ci:O_CH * (ci + 1)],
                                  po[:, 0:3 * M + 2])
            nc.sync.dma_start(O["o_all"][:, O_CH * ci:O_CH * (ci + 1)],
                              OSB[:, O_CH * ci:O_CH * (ci + 1)])
        # db_x / db_y columns: 8 tiny matmuls into one psum, one DVE copy
        for ci in range(4):
            mm(pbr[:, 10 + ci:11 + ci],
               u4x[:, ci * 128:(ci + 1) * 128].bitcast(F32), c[:].bitcast(F32),
               start=True, stop=True)
            mm(pbr[:, 14 + ci:15 + ci],
               u4y[:, ci * 128:(ci + 1) * 128].bitcast(F32), c[:].bitcast(F32),
               start=True, stop=True)
        nc.vector.tensor_copy(OSB[:, O_X:O_X + 8], pbr[:, 10:18])
        nc.sync.dma_start(O["o_all"][:, O_X:O_X + 8], OSB[:, O_X:O_X + 8])


def _split_multiwaits(nc):
    """Walrus codegen accepts a single sync-wait per instruction; Tile can
    emit several. Hoist all but the last wait onto same-engine NoOps
    inserted immediately before the over-subscribed instruction."""
    k = 0
    for fn in nc.m.functions:
        for bb in fn.blocks:
            il = bb.instructions
            out = []
            changed = False
            for inst in il:
                si = inst.sync_info
                waits = list(si.on_wait) if si and si.on_wait else []
                if len(waits) > 1:
                    changed = True
                    for w in waits[:-1]:
                        k += 1
                        nop = mybir.InstNoOp(name=f"I-wsplit-{k}")
                        nop.engine = inst.engine
                        nop.sync_info = mybir.SyncInfo(on_wait=[w], on_update=[])
                        nc.register_instruction(nop)
                        out.append(nop)
                    inst.sync_info = mybir.SyncInfo(
                        on_wait=[waits[-1]], on_update=list(si.on_update or []))
                out.append(inst)
            if changed:
                il[:] = out
                assert [i.name for i in bb.instructions] == [i.name for i in out]
    return k


def _build_nc():
    nc = bass.Bass()
    dp = nc.declare_dram_parameter
    D = {
        "pkB": dp("pkB", [2, CB], F32R, isOutput=False),
        "pkE": dp("pkE", [128, CE], F32, isOutput=False),
        "pkT": dp("pkT", [128, CT], F32R, isOutput=False),
        "pkV": dp("pkV", [128, CV], F32, isOutput=False),
    }
    O = {"o_all": dp("o_all", [128, CO], F32, isOutput=True)}
    with tile.TileContext(nc) as tc:
        _emit(nc, tc, D, O)
    _split_multiwaits(nc)
    return nc


def _prep_core_inputs(v_x, v_y, DT, cx, cy, brv, brmu, tr, lin_w):
    f = np.float32
    xv = np.concatenate([v_x, v_y], axis=2).reshape(B, DINV).astype(f)
    xt_x = cx.reshape(B, -1).astype(f)
    xt_y = cy.reshape(B, -1).astype(f)

    W1v = np.asarray(brv[0][0], f)
    W1mu = np.asarray(brmu[0][0], f)
    W1t = np.asarray(tr[0][0], f)
    perm = np.concatenate([np.arange(0, DINV, 2), np.arange(1, DINV, 2)])

    pkB = np.zeros((2, CB), f)
    pkB[0:2, B_W1T:B_W1T + 128] = W1t
    for k in range(4):
        pkB[0, B_BROWV + 128 * k:B_BROWV + 128 * (k + 1)] = np.asarray(brv[k][1], f)
        pkB[0, B_BROWM + 128 * k:B_BROWM + 128 * (k + 1)] = np.asarray(brmu[k][1], f)
    pkB[0, B_ONE] = 1.0

    pkE = np.zeros((128, CE), f)
    pkE[0:128, E_W1VA:E_W1VA + 128] = W1v[0:128]
    pkE[0:72, E_W1VB:E_W1VB + 128] = W1v[128:200]
    pkE[0:100, E_W1MU:E_W1MU + 128] = W1mu
    pkE[:, E_W2V:E_W2V + 128] = brv[1][0]
    pkE[:, E_W2MU:E_W2MU + 128] = brmu[1][0]
    for k in range(4):
        pkE[:, E_BT + k] = np.asarray(tr[k][1], f)
        pkE[:, E_BV + k] = np.asarray(brv[k][1], f)
        pkE[:, E_BMU + k] = np.asarray(brmu[k][1], f)
    pkE[:, E_LINW] = np.asarray(lin_w, f).reshape(J)

    pkT = np.zeros((128, CT), f)
    pkT[:, L_W2T:L_W2T + 128] = tr[1][0]
    pkT[:, L_W3T:L_W3T + 128] = tr[2][0]
    pkT[:, L_W4T:L_W4T + 128] = tr[3][0]
    pkT[:, L_W2TX:L_W2TX + 128] = np.asarray(tr[1][0], f) * W1t[0][:, None]
    pkT[:, L_W2TY:L_W2TY + 128] = np.asarray(tr[1][0], f) * W1t[1][:, None]
    pkV = np.zeros((128, CV), f)
    pkV[:, V_W3V:V_W3V + 128] = brv[2][0]
    pkV[:, V_W4V:V_W4V + 128] = brv[3][0]
    pkV[:, V_W3MU:V_W3MU + 128] = brmu[2][0]
    pkV[:, V_W4MU:V_W4MU + 128] = brmu[3][0]
    pkV[:, V_W1VT:V_W1VT + DINV] = W1v.T[:, perm]
    pkV[:, V_W1MUT:V_W1MUT + M] = W1mu.T

    in_maps = []
    for b in range(B):
        mB = pkB.copy()
        mB[0, B_XT:B_XT + NPT] = xt_x[b]
        mB[1, B_XT:B_XT + NPT] = xt_y[b]
        mE = pkE.copy()
        mE[0:128, E_XVA] = xv[b, 0:128]
        mE[0:72, E_XVB] = xv[b, 128:200]
        mE[0:100, E_XMU] = DT[b].astype(f)
        in_maps.append({"pkB": mB, "pkE": mE, "pkT": pkT, "pkV": pkV})
    return in_maps


def kernel(v_x_sampled, v_y_sampled, DT_sampled, coord_x, coord_y,
           brv_params, brmu_params, tr_params, lin_w):
    global _NC, LAST_RESULT
    a = np.asarray
    brv = [(a(W, dtype=np.float32), a(bb, dtype=np.float32)) for W, bb in brv_params]
    brmu = [(a(W, dtype=np.float32), a(bb, dtype=np.float32)) for W, bb in brmu_params]
    tr = [(a(W, dtype=np.float32), a(bb, dtype=np.float32)) for W, bb in tr_params]

    in_maps = _prep_core_inputs(
        a(v_x_sampled, dtype=np.float32), a(v_y_sampled, dtype=np.float32),
        a(DT_sampled, dtype=np.float32),
        a(coord_x, dtype=np.float32), a(coord_y, dtype=np.float32),
        brv, brmu, tr, a(lin_w, dtype=np.float32))

    if _NC is None:
        _NC = _build_nc()

    res = run_bass_kernel_spmd(_NC, in_maps, list(range(B)), trace=TRACE)
    LAST_RESULT = res

    def unpack(r):
        o = r["o_all"]
        vx = np.concatenate([o[:, O_CH * ci:O_CH * ci + M] for ci in range(4)])
        vy = np.concatenate([o[:, O_CH * ci + M:O_CH * ci + 2 * M]
                             for ci in range(4)])
        mu = np.concatenate([o[:, O_CH * ci + 2 * M:O_CH * ci + 3 * M]
                             for ci in range(4)])
        bas = np.concatenate([o[:, O_CH * ci + 3 * M] for ci in range(4)])
        x = np.concatenate([o[:, O_X + ci] for ci in range(4)])
        y = np.concatenate([o[:, O_Y + ci] for ci in range(4)])
        return bas, vx, vy, mu, x, y

    outs = [unpack(r) for r in res.results]
    basis = np.stack([o[0].reshape(E, I, 1) for o in outs])
    db_vx = np.stack([o[1].reshape(E, I, M, 1) for o in outs])
    db_vy = np.stack([o[2].reshape(E, I, M, 1) for o in outs])
    db_mu = np.stack([o[3].reshape(E, I, M, 1) for o in outs])
    db_x = np.stack([o[4].reshape(E, I, 1) for o in outs])
    db_y = np.stack([o[5].reshape(E, I, 1) for o in outs])
    return (basis, db_vx, db_vy, db_mu, db_x, db_y)
